# revision 20
# baseline (speedup 1.0000x reference)
"""GNN message passing (GraphConv x3 + TopKPooling + MLP head) on 8 trn2 cores.

Strategy: shard the 128 graphs across 8 cores (16 graphs/core). On host,
convert each graph's edge list into a dense 512x512 adjacency-count matrix
AT[s, d] = #edges s->d (exact in fp16). On device, segment-sum message
passing becomes dense PE matmuls (fp16 operands, f32 PSUM accumulate):

  A-step:  U_l = (A @ h_l)^T : stat = h_l (normal [node, feat]), mov = AT
           -> PSUM [feat, node] ("T-land")
  W-step(a): H_{l+1} = relu(Wst_l^T @ [H_l; U_l] + b_l)    (T-land)
  W-step(b): h_{l+1} = relu([H_l; U_l]^T-stat @ Wst_l-mov) (normal)
           -- same product with swapped stationary/moving roles; this yields
              both layouts without any transpose instruction.

Pools: mean pools ride the Act-engine relu drains (accum_out); max pools are
DVE free-dim reduces. Top-k keeps the 410 best scores per graph: the 103rd
smallest score is found with 13 rounds of DVE max8/match_replace on negated
scores, then masked pooling (mask/tanh rows broadcast to 128 partitions via
a DRAM-replicated DMA; elementwise on GPSIMD, reduces on DVE).

Hardware quirks handled: SBUF partition bases must be in {0,32,64,96} for
every instruction, so per-graph score rows are accumulated into a [4, 512]
batch tile via one-cold-column stationary matmuls; the walrus build also
rejects dma_transpose / gpsimd-extended ops / tensor_tensor_reduce, none of
which are used.
"""

import re

import numpy as np

NCORES = 8
G = 16          # graphs per core
N = 512         # nodes per graph
D = 256         # embed dim
E = 8192        # edges per graph
K = 410         # top-k kept per graph (ceil(0.8 * 512))
TB = 4          # topk batch size

F16 = np.float16
NEG_BIG = -60000.0  # fits fp16; dominates any |h * tanh| product


_PATCHED = False


def _apply_tile_patch():
    """walrus rejects >1 sem-wait on the final SP Drain: split into nops."""
    global _PATCHED
    if _PATCHED:
        return
    import bass_rust
    from concourse.tile import TileContext
    from concourse.vector_clock import ScopedClock

    def _patched(self, tick_clock, wait_clock):
        vals = [int(x) for x in re.findall(r"\d+", repr(tick_clock.global_clock))]
        for i, v in enumerate(vals):
            if v <= 0:
                continue
            single = [0] * len(vals)
            single[i] = v
            nop_inst = self.nc.sync.nop(nofuse=True, hint=f"split_drain_{i}")
            wait_clock.add_sem_waits(
                nop_inst.ins, ScopedClock({None: bass_rust.VectorClock(single)})
            )
        self.nc.sync.drain()
        self.nc.all_engine_barrier()
        assert self.sems is not None
        popped = self.nc._tile_sem_poison_stack.pop()
        assert popped is self._sem_poison
        self.nc.clear_and_free_semaphores(list(self.sems.allocated().values()))
        self.nc.all_engine_barrier()

    TileContext._drain_and_barrier = _patched
    _PATCHED = True


def _split_multi_waits(nc):
    """walrus allows only one sem-wait per instruction: hoist extras onto
    injected same-engine nops placed immediately before the instruction
    (per-engine program order makes the earlier wait a safe strengthening)."""
    import bass_rust
    import concourse.mybir as mybir

    n = 0
    for fn in nc.m.functions:
        for bb in fn.blocks:
            out = []
            for inst in bb.instructions:
                si = inst.sync_info
                if si and si.on_wait and len(si.on_wait) > 1:
                    waits = list(si.on_wait)
                    for w in waits[:-1]:
                        nop = bass_rust.InstNoOp(
                            name=f"I-waitsplit-{nc.next_id()}", ins=[], outs=[])
                        nop.engine = inst.engine
                        nop.sync_info = mybir.SyncInfo(on_wait=[w], on_update=[])
                        nc.register_instruction(nop, overwrite=True)
                        out.append(nop)
                        n += 1
                    si.on_wait = [waits[-1]]
                out.append(inst)
            bb.instructions = out
    return n


def build_program(has_bias=False):
    _apply_tile_patch()
    import concourse.bass as bass
    import concourse.mybir as mybir
    from concourse.tile import TileContext

    dt = mybir.dt
    f32 = dt.float32
    f16 = dt.float16
    Alu = mybir.AluOpType
    Act = mybir.ActivationFunctionType
    AX = mybir.AxisListType.X

    nc = bass.Bass()
    dp = nc.declare_dram_parameter
    at_d = dp("at", [G, N, N], f16, isOutput=False)
    xs_d = dp("xs", [G, N, 4], f16, isOutput=False)
    xt_d = dp("xt", [G, 4, N], f16, isOutput=False)
    ws1_d = dp("ws1", [4, D], f16, isOutput=False)
    wr1_d = dp("wr1", [4, D], f16, isOutput=False)
    wst2_d = dp("wst2", [2 * D, D], f16, isOutput=False)
    wst3_d = dp("wst3", [2 * D, D], f16, isOutput=False)
    b1_d = dp("b1", [D, 1], f32, isOutput=False)
    b2_d = dp("b2", [D, 1], f32, isOutput=False)
    b3_d = dp("b3", [D, 1], f32, isOutput=False)
    b1r_d = dp("b1r", [1, D], f16, isOutput=False)
    b2r_d = dp("b2r", [1, D], f16, isOutput=False)
    pmat_d = dp("pmat", [128, 2, TB, TB], f16, isOutput=False)
    prow_d = dp("prow", [1, D], f32, isOutput=False)
    mw1_d = dp("mw1", [2 * D, D], f16, isOutput=False)
    mw2_d = dp("mw2", [D, D // 2], f16, isOutput=False)
    mw3_d = dp("mw3", [D // 2, 1], f16, isOutput=False)
    c1_d = dp("c1", [D, 1], f32, isOutput=False)
    c2_d = dp("c2", [D // 2, 1], f32, isOutput=False)
    c3_d = dp("c3", [1, 1], f32, isOutput=False)
    y_d = dp("y", [1, G], f32, isOutput=True)

    # DRAM scratch for broadcast bounces
    t3_dr = [nc.dram_tensor(f"t3d{b}", [TB, 3, N], f16) for b in range(G // TB)]
    inv_dr = nc.dram_tensor("invd", [1, 1], f32)

    with TileContext(nc) as tc:
        with (
            tc.tile_pool(name="const", bufs=1) as cpool,
            tc.tile_pool(name="at", bufs=3) as atpool,
            tc.tile_pool(name="xin", bufs=3) as xpool,
            tc.tile_pool(name="h1", bufs=2) as h1pool,
            tc.tile_pool(name="h2", bufs=2) as h2pool,
            tc.tile_pool(name="h3", bufs=8) as h3pool,
            tc.tile_pool(name="hn", bufs=3) as hnpool,
            tc.tile_pool(name="usb", bufs=2) as upool,
            tc.tile_pool(name="stats", bufs=1) as spool,
            tc.tile_pool(name="tkb", bufs=2) as tkpool,
            tc.tile_pool(name="scratch", bufs=4) as scpool,
            tc.tile_pool(name="psU", bufs=2, space="PSUM") as psU,
            tc.tile_pool(name="psH", bufs=2, space="PSUM") as psH,
            tc.tile_pool(name="psB", bufs=1, space="PSUM") as psB,
            tc.tile_pool(name="psS", bufs=2, space="PSUM") as psS,
        ):
            # ---- one-time constants -------------------------------------
            wst2_sb = cpool.tile([128, 4, D], f16)
            nc.sync.dma_start(out=wst2_sb, in_=wst2_d.rearrange("(a p) m -> p a m", p=128))
            wst3_sb = cpool.tile([128, 4, D], f16)
            nc.sync.dma_start(out=wst3_sb, in_=wst3_d.rearrange("(a p) m -> p a m", p=128))
            ws1_sb = cpool.tile([4, D], f16)
            nc.sync.dma_start(out=ws1_sb, in_=ws1_d[:, :])
            wr1_sb = cpool.tile([4, D], f16)
            nc.sync.dma_start(out=wr1_sb, in_=wr1_d[:, :])
            b_sb = []
            for bd in (b1_d, b2_d, b3_d):
                t = cpool.tile([128, 2, 1], f32, name=f"b_sb_{bd.name}")
                nc.sync.dma_start(out=t, in_=bd.rearrange("(a p) o -> p a o", p=128))
                b_sb.append(t)
            if has_bias:
                b1r_sb = cpool.tile([1, D], f16)
                nc.sync.dma_start(out=b1r_sb, in_=b1r_d[:, :])
                b2r_sb = cpool.tile([1, D], f16)
                nc.sync.dma_start(out=b2r_sb, in_=b2r_d[:, :])
                ones1 = cpool.tile([1, N], f16)
                nc.vector.memset(ones1, 1.0)
            pmat_sb = cpool.tile([128, 2, TB, TB], f16)
            nc.sync.dma_start(out=pmat_sb, in_=pmat_d[:, :, :, :])
            prow_sb = cpool.tile([1, D], f32)
            nc.sync.dma_start(out=prow_sb, in_=prow_d[:, :])
            mw1_sb = cpool.tile([128, 4, D], f16)
            nc.sync.dma_start(out=mw1_sb, in_=mw1_d.rearrange("(a p) m -> p a m", p=128))
            mw2_sb = cpool.tile([128, 2, D // 2], f16)
            nc.sync.dma_start(out=mw2_sb, in_=mw2_d.rearrange("(a p) m -> p a m", p=128))
            mw3_sb = cpool.tile([128, 1], f16)
            nc.sync.dma_start(out=mw3_sb, in_=mw3_d[:, :])
            c1_sb = cpool.tile([128, 2, 1], f32)
            nc.sync.dma_start(out=c1_sb, in_=c1_d.rearrange("(a p) o -> p a o", p=128))
            c2_sb = cpool.tile([128, 1], f32)
            nc.sync.dma_start(out=c2_sb, in_=c2_d[:, :])
            c3_sb = cpool.tile([1, 1], f32)
            nc.sync.dma_start(out=c3_sb, in_=c3_d[:, :])

            # inv = 1 / ||p||  (the reference's +1e-16 is numerically inert)
            pnorm2 = spool.tile([1, 1], f32)
            sq_scratch = spool.tile([1, D], f32)
            nc.scalar.activation(sq_scratch, prow_sb, Act.Square, accum_out=pnorm2)
            pnorm = spool.tile([1, 1], f32)
            nc.scalar.activation(pnorm, pnorm2, Act.Sqrt, bias=0.0, scale=1.0)
            inv1 = spool.tile([1, 1], f32)
            nc.vector.reciprocal(inv1, pnorm)
            nc.sync.dma_start(out=inv_dr[:, :], in_=inv1)
            invB = spool.tile([TB, 1], f32)
            nc.sync.dma_start(out=invB, in_=inv_dr.broadcast_to([TB, 1]))

            # ---- per-graph accumulators ---------------------------------
            zmax = [[spool.tile([128, G], f32, tag=f"zmax{l}{fh}", name=f"zmax{l}{fh}")
                     for fh in range(2)] for l in range(3)]
            zsum = [[spool.tile([128, G], f32, tag=f"zsum{l}{fh}", name=f"zsum{l}{fh}")
                     for fh in range(2)] for l in range(3)]

            h3_keep = []
            sc_ps = None

            for g in range(G):
                b = g // TB
                j = g % TB

                # ---- input DMAs ----------------------------------------
                at_sb = atpool.tile([128, 4, N], f16, tag="at")
                nc.scalar.dma_start(out=at_sb, in_=at_d[g].rearrange("(a p) n -> p a n", p=128))
                xs_sb = xpool.tile([128, 4, 4], f16, tag="xs")
                nc.sync.dma_start(out=xs_sb, in_=xs_d[g].rearrange("(a p) f -> p a f", p=128))
                xt_sb = xpool.tile([4, N], f16, tag="xt")
                nc.sync.dma_start(out=xt_sb, in_=xt_d[g])

                # ---- layer 1 -------------------------------------------
                u1_ps = psS.tile([4, N], f32, tag="psmall")
                for nb in range(4):
                    nc.tensor.matmul(u1_ps, xs_sb[:, nb, :], at_sb[:, nb, :],
                                     start=(nb == 0), stop=(nb == 3))
                u1_sb = xpool.tile([4, N], f16, tag="u1")
                nc.scalar.copy(u1_sb, u1_ps)

                # (a) T-land H1
                H1 = h1pool.tile([128, 2, N], f16, tag="H1")
                for mh in range(2):
                    h_ps = psH.tile([128, N], f32, tag="psh")
                    nc.tensor.matmul(h_ps, ws1_sb[:, mh * 128:(mh + 1) * 128], xt_sb,
                                     start=True, stop=False)
                    nc.tensor.matmul(h_ps, wr1_sb[:, mh * 128:(mh + 1) * 128], u1_sb,
                                     start=False, stop=True)
                    nc.scalar.activation(H1[:, mh, :], h_ps, Act.Relu,
                                         bias=b_sb[0][:, mh, :], scale=1.0,
                                         accum_out=zsum[0][mh][:, g:g + 1])
                    nc.vector.reduce_max(zmax[0][mh][:, g:g + 1], H1[:, mh, :], axis=AX)

                # (b) normal h1 [node, feat] for the L2 A-step stationary
                hb_ps = psB.tile([128, 4, D], f32, tag="psb")
                for nb in range(4):
                    sl = slice(nb * 128, (nb + 1) * 128)
                    nc.tensor.matmul(hb_ps[:, nb, :], xt_sb[:, sl], ws1_sb,
                                     start=True, stop=False)
                    nc.tensor.matmul(hb_ps[:, nb, :], u1_sb[:, sl], wr1_sb,
                                     start=False, stop=(not has_bias))
                    if has_bias:
                        nc.tensor.matmul(hb_ps[:, nb, :], ones1[:, sl], b1r_sb,
                                         start=False, stop=True)
                h1n = hnpool.tile([128, 4, D], f16, tag="hn")
                for nb in range(4):
                    nc.scalar.activation(h1n[:, nb, :], hb_ps[:, nb, :], Act.Relu,
                                         bias=0.0, scale=1.0)

                # ---- layer 2 -------------------------------------------
                u_sb = upool.tile([128, 2, N], f16, tag="u")
                for fh in range(2):
                    u_ps = psU.tile([128, N], f32, tag="psu")
                    for nb in range(4):
                        nc.tensor.matmul(u_ps, h1n[:, nb, fh * 128:(fh + 1) * 128],
                                         at_sb[:, nb, :], start=(nb == 0), stop=(nb == 3))
                    nc.scalar.copy(u_sb[:, fh, :], u_ps)

                H2 = h2pool.tile([128, 2, N], f16, tag="H2")
                for mh in range(2):
                    h_ps = psH.tile([128, N], f32, tag="psh")
                    for kb in range(4):
                        mov = H1[:, kb, :] if kb < 2 else u_sb[:, kb - 2, :]
                        nc.tensor.matmul(h_ps, wst2_sb[:, kb, mh * 128:(mh + 1) * 128],
                                         mov, start=(kb == 0), stop=(kb == 3))
                    nc.scalar.activation(H2[:, mh, :], h_ps, Act.Relu,
                                         bias=b_sb[1][:, mh, :], scale=1.0,
                                         accum_out=zsum[1][mh][:, g:g + 1])
                    nc.vector.reduce_max(zmax[1][mh][:, g:g + 1], H2[:, mh, :], axis=AX)

                hb_ps2 = psB.tile([128, 4, D], f32, tag="psb")
                for nb in range(4):
                    sl = slice(nb * 128, (nb + 1) * 128)
                    for kb in range(4):
                        stat = H1[:, kb, sl] if kb < 2 else u_sb[:, kb - 2, sl]
                        nc.tensor.matmul(hb_ps2[:, nb, :], stat, wst2_sb[:, kb, :],
                                         start=(kb == 0),
                                         stop=(kb == 3 and not has_bias))
                    if has_bias:
                        nc.tensor.matmul(hb_ps2[:, nb, :], ones1[:, sl], b2r_sb,
                                         start=False, stop=True)
                h2n = hnpool.tile([128, 4, D], f16, tag="hn")
                for nb in range(4):
                    nc.scalar.activation(h2n[:, nb, :], hb_ps2[:, nb, :], Act.Relu,
                                         bias=0.0, scale=1.0)

                # ---- layer 3 (T-land only) ------------------------------
                u_sb3 = upool.tile([128, 2, N], f16, tag="u")
                for fh in range(2):
                    u_ps = psU.tile([128, N], f32, tag="psu")
                    for nb in range(4):
                        nc.tensor.matmul(u_ps, h2n[:, nb, fh * 128:(fh + 1) * 128],
                                         at_sb[:, nb, :], start=(nb == 0), stop=(nb == 3))
                    nc.scalar.copy(u_sb3[:, fh, :], u_ps)

                H3 = h3pool.tile([128, 2, N], f16, tag="H3")
                h3_keep.append(H3)
                for mh in range(2):
                    h_ps = psH.tile([128, N], f32, tag="psh")
                    for kb in range(4):
                        mov = H2[:, kb, :] if kb < 2 else u_sb3[:, kb - 2, :]
                        nc.tensor.matmul(h_ps, wst3_sb[:, kb, mh * 128:(mh + 1) * 128],
                                         mov, start=(kb == 0), stop=(kb == 3))
                    nc.scalar.activation(H3[:, mh, :], h_ps, Act.Relu,
                                         bias=b_sb[2][:, mh, :], scale=1.0)

                # ---- score: accumulate row j of the batch tile ----------
                # stationary column j = p_attn, other columns zero, so graph
                # g's scores land in psum row j while other rows add zero.
                if j == 0:
                    sc_ps = psS.tile([TB, N], f32, tag="psmall", name=f"sc_ps{b}")
                for fh in range(2):
                    nc.tensor.matmul(sc_ps, pmat_sb[:, fh, j, :], H3[:, fh, :],
                                     start=(j == 0 and fh == 0),
                                     stop=(j == TB - 1 and fh == 1))

                # ---- topk + x3 pooling per batch ------------------------
                if j == TB - 1:
                    scores = tkpool.tile([TB, N], f32, tag="scores")
                    nc.scalar.copy(scores, sc_ps)
                    tneg = [tkpool.tile([TB, N], f32, tag=f"tneg{i}", name=f"tneg{i}_{b}")
                            for i in range(2)]
                    m8 = tkpool.tile([TB, 8], f32, tag="m8")
                    vthr = tkpool.tile([TB, 1], f32, tag="vthr")
                    t3 = tkpool.tile([TB, 3, N], f16, tag="t3")

                    nc.vector.tensor_scalar_mul(tneg[0], scores, -1.0)
                    cur = 0
                    for r in range(12):
                        nc.vector.max(m8, tneg[cur])
                        nc.vector.match_replace(tneg[1 - cur], m8, tneg[cur], -1e30)
                        cur = 1 - cur
                    nc.vector.max(m8, tneg[cur])
                    # threshold = 103rd smallest score = -(m8 col 6)
                    nc.vector.tensor_scalar_mul(vthr, m8[:, 6:7], -1.0)
                    # plane 0: tanh(s/||p||); plane 1: w = tanh*mask; plane 2: negmask
                    nc.scalar.activation(t3[:, 0, :], scores, Act.Tanh,
                                         bias=0.0, scale=invB)
                    nc.vector.tensor_scalar(t3[:, 1, :], scores, vthr, None, Alu.is_ge)
                    nc.vector.tensor_scalar(t3[:, 2, :], t3[:, 1, :], -NEG_BIG,
                                            NEG_BIG, Alu.mult, Alu.add)
                    nc.vector.tensor_tensor(out=t3[:, 1, :], in0=t3[:, 0, :],
                                            in1=t3[:, 1, :], op=Alu.mult)
                    nc.sync.dma_start(out=t3_dr[b][:, :, :], in_=t3)

                    for jj in range(TB):
                        gg = b * TB + jj
                        H3g = h3_keep[gg]
                        bc = scpool.tile([128, 3, N], f16, tag="bc")
                        nc.sync.dma_start(
                            out=bc,
                            in_=t3_dr[b][jj:jj + 1].broadcast_to([128, 3, N]))
                        for fh in range(2):
                            p1 = scpool.tile([128, N], f32, tag="p1")
                            nc.gpsimd.tensor_tensor(out=p1, in0=H3g[:, fh, :],
                                                    in1=bc[:, 0, :], op=Alu.mult)
                            p1b = scpool.tile([128, N], f32, tag="p1b")
                            nc.gpsimd.tensor_tensor(out=p1b, in0=p1,
                                                    in1=bc[:, 2, :], op=Alu.add)
                            nc.vector.reduce_max(zmax[2][fh][:, gg:gg + 1], p1b, axis=AX)
                            p2 = scpool.tile([128, N], f32, tag="p2")
                            nc.vector.tensor_tensor(out=p2, in0=H3g[:, fh, :],
                                                    in1=bc[:, 1, :], op=Alu.mult)
                            nc.vector.reduce_sum(zsum[2][fh][:, gg:gg + 1], p2, axis=AX)

            # ---- assemble z and run the MLP head ------------------------
            zmx = [scpool.tile([128, G], f16, tag=f"zmx{fh}", name=f"zmx{fh}")
                   for fh in range(2)]
            zmn = [scpool.tile([128, G], f16, tag=f"zmn{fh}", name=f"zmn{fh}")
                   for fh in range(2)]
            for fh in range(2):
                ztmp = scpool.tile([128, G], f32, tag="ztmp")
                nc.vector.tensor_add(ztmp, zmax[0][fh], zmax[1][fh])
                nc.vector.tensor_add(zmx[fh], ztmp, zmax[2][fh])
                ztmp2 = scpool.tile([128, G], f32, tag="ztmp2")
                nc.vector.tensor_add(ztmp2, zsum[0][fh], zsum[1][fh])
                nc.vector.tensor_scalar_mul(ztmp2, ztmp2, 1.0 / N)
                nc.vector.scalar_tensor_tensor(
                    out=zmn[fh], in0=zsum[2][fh], scalar=1.0 / K, in1=ztmp2,
                    op0=Alu.mult, op1=Alu.add)

            z_tiles = [zmx[0], zmx[1], zmn[0], zmn[1]]
            zz1 = scpool.tile([128, 2, G], f16, tag="zz1")
            for mh in range(2):
                mlp_ps = psH.tile([128, G], f32, tag="psh")
                for kb in range(4):
                    nc.tensor.matmul(mlp_ps, mw1_sb[:, kb, mh * 128:(mh + 1) * 128],
                                     z_tiles[kb], start=(kb == 0), stop=(kb == 3))
                nc.scalar.activation(zz1[:, mh, :], mlp_ps, Act.Relu,
                                     bias=c1_sb[:, mh, :], scale=1.0)
            zz2 = scpool.tile([128, G], f16, tag="zz2")
            mlp_ps2 = psH.tile([128, G], f32, tag="psh")
            for kb in range(2):
                nc.tensor.matmul(mlp_ps2, mw2_sb[:, kb, :], zz1[:, kb, :],
                                 start=(kb == 0), stop=(kb == 1))
            nc.scalar.activation(zz2, mlp_ps2, Act.Relu, bias=c2_sb, scale=1.0)
            y_ps = psS.tile([1, G], f32, tag="psmall")
            nc.tensor.matmul(y_ps, mw3_sb, zz2, start=True, stop=True)
            y_sb = scpool.tile([1, G], f32, tag="ysb")
            nc.scalar.activation(y_sb, y_ps, Act.Sigmoid, bias=c3_sb, scale=1.0)
            nc.sync.dma_start(out=y_d[:, :], in_=y_sb)

    _split_multi_waits(nc)
    return nc


# ---------------------------------------------------------------------------
# Host-side packing
# ---------------------------------------------------------------------------

def pack_inputs(inputs):
    x = np.asarray(inputs["x"], np.float32)
    src = np.asarray(inputs["src"]).astype(np.int64)
    dst = np.asarray(inputs["dst"]).astype(np.int64)

    def hf(a):
        return np.ascontiguousarray(np.asarray(a, np.float32).astype(F16))

    ws1 = hf(inputs["Ws1"]); wr1 = hf(inputs["Wr1"])
    wst2 = hf(np.concatenate([inputs["Ws2"], inputs["Wr2"]], axis=0))
    wst3 = hf(np.concatenate([inputs["Ws3"], inputs["Wr3"]], axis=0))
    b1 = np.asarray(inputs["b1"], np.float32).reshape(D, 1)
    b2 = np.asarray(inputs["b2"], np.float32).reshape(D, 1)
    b3 = np.asarray(inputs["b3"], np.float32).reshape(D, 1)
    p = np.asarray(inputs["p_attn"], np.float32)
    pmat = np.zeros((128, 2, TB, TB), np.float32)
    for j in range(TB):
        pmat[:, 0, j, j] = p[:128]
        pmat[:, 1, j, j] = p[128:]
    prow = np.ascontiguousarray(p.reshape(1, D))
    mw1 = hf(inputs["W1"]); mw2 = hf(inputs["W2"]); mw3 = hf(inputs["W3"])
    c1 = np.asarray(inputs["c1"], np.float32).reshape(D, 1)
    c2 = np.asarray(inputs["c2"], np.float32).reshape(D // 2, 1)
    c3 = np.asarray(inputs["c3"], np.float32).reshape(1, 1)

    shared = dict(ws1=ws1, wr1=wr1, wst2=wst2, wst3=wst3, b1=b1, b2=b2, b3=b3,
                  b1r=hf(b1.reshape(1, D)), b2r=hf(b2.reshape(1, D)),
                  pmat=hf(pmat), prow=prow, mw1=mw1, mw2=mw2, mw3=mw3,
                  c1=c1, c2=c2, c3=c3)

    in_maps = []
    for c in range(NCORES):
        g0 = c * G
        at = np.empty((G, N, N), F16)
        xs = np.empty((G, N, 4), F16)
        xt = np.empty((G, 4, N), F16)
        for g in range(G):
            gg = g0 + g
            s = src[gg * E:(gg + 1) * E] - gg * N
            d_ = dst[gg * E:(gg + 1) * E] - gg * N
            cnt = np.bincount(s * N + d_, minlength=N * N).reshape(N, N)
            at[g] = cnt.astype(F16)
            xg = x[gg * N:(gg + 1) * N]
            xs[g] = xg.astype(F16)
            xt[g] = np.ascontiguousarray(xg.T).astype(F16)
        in_maps.append(dict(at=at, xs=xs, xt=xt, **shared))
    return in_maps


def has_nonzero_bias(inputs):
    return any(np.any(np.asarray(inputs[k]) != 0) for k in ("b1", "b2"))


def kernel(**inputs):
    from concourse.bass_utils import run_bass_kernel_spmd

    nc = build_program(has_bias=has_nonzero_bias(inputs))
    in_maps = pack_inputs(inputs)
    res = run_bass_kernel_spmd(nc, in_maps, list(range(NCORES)))
    y = np.concatenate([np.asarray(res.results[c]["y"], np.float32).reshape(-1)
                        for c in range(NCORES)])
    return y


# revision 33
# speedup vs baseline: 1.0328x; 1.0328x over previous
"""GNN message passing (GraphConv x3 + TopKPooling + MLP head) on 8 trn2 cores.

Strategy: shard the 128 graphs across 8 cores (16 graphs/core). On host,
convert each graph's edge list into a dense 512x512 adjacency-count matrix
AT[s, d] = #edges s->d (exact in fp16). On device, segment-sum message
passing becomes dense PE matmuls (fp16 operands, f32 PSUM accumulate):

  A-step:  U_l = (A @ h_l)^T : stat = h_l (normal [node, feat]), mov = AT
           -> PSUM [feat, node] ("T-land")
  W-step(a): H_{l+1} = relu(Wst_l^T @ [H_l; U_l] + b_l)    (T-land)
  W-step(b): h_{l+1} = relu([H_l; U_l]^T-stat @ Wst_l-mov) (normal)
           -- same product with swapped stationary/moving roles; this yields
              both layouts without any transpose instruction.

Pools: mean pools ride the Act-engine relu drains (accum_out); max pools are
DVE free-dim reduces. Top-k keeps the 410 best scores per graph: the 103rd
smallest score is found with 13 rounds of DVE max8/match_replace on negated
scores, then masked pooling (mask/tanh rows broadcast to 128 partitions via
a DRAM-replicated DMA; elementwise on GPSIMD, reduces on DVE).

Hardware quirks handled: SBUF partition bases must be in {0,32,64,96} for
every instruction, so per-graph score rows are accumulated into a [4, 512]
batch tile via one-cold-column stationary matmuls; the walrus build also
rejects dma_transpose / gpsimd-extended ops / tensor_tensor_reduce, none of
which are used.
"""

import re

import numpy as np

NCORES = 8
G = 16          # graphs per core
N = 512         # nodes per graph
D = 256         # embed dim
E = 8192        # edges per graph
K = 410         # top-k kept per graph (ceil(0.8 * 512))
BATCHES = [(0, 7), (7, 6), (13, 3)]  # sized so each topk+x3 overlaps remaining PE work
TBMAX = 7

F16 = np.float16
# h3 >= 0 (post-relu) and the top-410 always contains a node with tanh >= 0,
# so masked-max == max(H3*w) with excluded entries zeroed by the mask.


_PATCHED = False


def _apply_tile_patch():
    """walrus rejects >1 sem-wait on the final SP Drain: split into nops."""
    global _PATCHED
    if _PATCHED:
        return
    import bass_rust
    from concourse.tile import TileContext
    from concourse.vector_clock import ScopedClock

    def _patched(self, tick_clock, wait_clock):
        vals = [int(x) for x in re.findall(r"\d+", repr(tick_clock.global_clock))]
        for i, v in enumerate(vals):
            if v <= 0:
                continue
            single = [0] * len(vals)
            single[i] = v
            nop_inst = self.nc.sync.nop(nofuse=True, hint=f"split_drain_{i}")
            wait_clock.add_sem_waits(
                nop_inst.ins, ScopedClock({None: bass_rust.VectorClock(single)})
            )
        self.nc.sync.drain()
        self.nc.all_engine_barrier()
        assert self.sems is not None
        popped = self.nc._tile_sem_poison_stack.pop()
        assert popped is self._sem_poison
        self.nc.clear_and_free_semaphores(list(self.sems.allocated().values()))
        self.nc.all_engine_barrier()

    TileContext._drain_and_barrier = _patched
    _PATCHED = True


def _split_multi_waits(nc):
    """walrus allows only one sem-wait per instruction: hoist extras onto
    injected same-engine nops placed immediately before the instruction
    (per-engine program order makes the earlier wait a safe strengthening)."""
    import bass_rust
    import concourse.mybir as mybir

    n = 0
    for fn in nc.m.functions:
        for bb in fn.blocks:
            out = []
            for inst in bb.instructions:
                si = inst.sync_info
                if si and si.on_wait and len(si.on_wait) > 1:
                    waits = list(si.on_wait)
                    for w in waits[:-1]:
                        nop = bass_rust.InstNoOp(
                            name=f"I-waitsplit-{nc.next_id()}", ins=[], outs=[])
                        nop.engine = inst.engine
                        nop.sync_info = mybir.SyncInfo(on_wait=[w], on_update=[])
                        nc.register_instruction(nop, overwrite=True)
                        out.append(nop)
                        n += 1
                    si.on_wait = [waits[-1]]
                out.append(inst)
            bb.instructions = out
    return n


def build_program(has_bias=False):
    _apply_tile_patch()
    import concourse.bass as bass
    import concourse.mybir as mybir
    from concourse.tile import TileContext

    dt = mybir.dt
    f32 = dt.float32
    f16 = dt.float16
    Alu = mybir.AluOpType
    Act = mybir.ActivationFunctionType
    AX = mybir.AxisListType.X

    nc = bass.Bass()
    dp = nc.declare_dram_parameter
    at_d = dp("at", [G, N, N], f16, isOutput=False)
    xs_d = dp("xs", [G, N, 4], f16, isOutput=False)
    xt_d = dp("xt", [G, 4, N], f16, isOutput=False)
    ws1_d = dp("ws1", [4, D], f16, isOutput=False)
    wr1_d = dp("wr1", [4, D], f16, isOutput=False)
    wst2_d = dp("wst2", [2 * D, D], f16, isOutput=False)
    wst3_d = dp("wst3", [2 * D, D], f16, isOutput=False)
    b1_d = dp("b1", [D, 1], f32, isOutput=False)
    b2_d = dp("b2", [D, 1], f32, isOutput=False)
    b3_d = dp("b3", [D, 1], f32, isOutput=False)
    b1r_d = dp("b1r", [1, D], f16, isOutput=False)
    b2r_d = dp("b2r", [1, D], f16, isOutput=False)
    pmat_d = dp("pmat", [128, 2, G, TBMAX], f16, isOutput=False)
    prow_d = dp("prow", [1, D], f32, isOutput=False)
    mw1_d = dp("mw1", [2 * D, D], f16, isOutput=False)
    mw2_d = dp("mw2", [D, D // 2], f16, isOutput=False)
    mw3_d = dp("mw3", [D // 2, 1], f16, isOutput=False)
    c1_d = dp("c1", [D, 1], f32, isOutput=False)
    c2_d = dp("c2", [D // 2, 1], f32, isOutput=False)
    c3_d = dp("c3", [1, 1], f32, isOutput=False)
    y_d = dp("y", [1, G], f32, isOutput=True)

    # DRAM scratch for broadcast bounces
    t3_dr = [nc.dram_tensor(f"t3d{i}", [sz, N], f16) for i, (s, sz) in enumerate(BATCHES)]
    inv_dr = nc.dram_tensor("invd", [1, 1], f32)

    with TileContext(nc) as tc:
        with (
            tc.tile_pool(name="const", bufs=1) as cpool,
            tc.tile_pool(name="at", bufs=3) as atpool,
            tc.tile_pool(name="xin", bufs=3) as xpool,
            tc.tile_pool(name="h1", bufs=5) as h1pool,
            tc.tile_pool(name="h2", bufs=5) as h2pool,
            tc.tile_pool(name="h3", bufs=16) as h3pool,
            tc.tile_pool(name="hn", bufs=3) as hnpool,
            tc.tile_pool(name="usb", bufs=2) as upool,
            tc.tile_pool(name="stats", bufs=1) as spool,
            tc.tile_pool(name="tkb", bufs=2) as tkpool,
            tc.tile_pool(name="scratch", bufs=4) as scpool,
            tc.tile_pool(name="psU", bufs=2, space="PSUM") as psU,
            tc.tile_pool(name="psH", bufs=2, space="PSUM") as psH,
            tc.tile_pool(name="psB", bufs=1, space="PSUM") as psB,
            tc.tile_pool(name="psS", bufs=2, space="PSUM") as psS,
        ):
            # ---- prefetch first graphs' inputs before heavy consts ------
            gdata = {}

            def load_graph(g):
                a = atpool.tile([128, 4, N], f16, tag="at", name=f"at{g}")
                nc.scalar.dma_start(out=a, in_=at_d[g].rearrange("(a p) n -> p a n", p=128))
                xs = xpool.tile([128, 4, 4], f16, tag="xs", name=f"xs{g}")
                nc.sync.dma_start(out=xs, in_=xs_d[g].rearrange("(a p) f -> p a f", p=128))
                xt = xpool.tile([4, N], f16, tag="xt", name=f"xt{g}")
                nc.sync.dma_start(out=xt, in_=xt_d[g])
                gdata[g] = (a, xs, xt)

            ws1_sb = cpool.tile([4, D], f16)
            nc.sync.dma_start(out=ws1_sb, in_=ws1_d[:, :])
            wr1_sb = cpool.tile([4, D], f16)
            nc.sync.dma_start(out=wr1_sb, in_=wr1_d[:, :])
            for _pg in range(3):
                load_graph(_pg)

            # ---- one-time constants -------------------------------------
            wst2_sb = cpool.tile([128, 4, D], f16)
            nc.sync.dma_start(out=wst2_sb, in_=wst2_d.rearrange("(a p) m -> p a m", p=128))
            wst3_sb = cpool.tile([128, 4, D], f16)
            nc.sync.dma_start(out=wst3_sb, in_=wst3_d.rearrange("(a p) m -> p a m", p=128))
            b_sb = []
            for bd in (b1_d, b2_d, b3_d):
                t = cpool.tile([128, 2, 1], f32, name=f"b_sb_{bd.name}")
                nc.sync.dma_start(out=t, in_=bd.rearrange("(a p) o -> p a o", p=128))
                b_sb.append(t)
            if has_bias:
                b1r_sb = cpool.tile([1, D], f16)
                nc.sync.dma_start(out=b1r_sb, in_=b1r_d[:, :])
                b2r_sb = cpool.tile([1, D], f16)
                nc.sync.dma_start(out=b2r_sb, in_=b2r_d[:, :])
                ones1 = cpool.tile([1, N], f16)
                nc.vector.memset(ones1, 1.0)
            pmat_sb = cpool.tile([128, 2, G, TBMAX], f16)
            nc.sync.dma_start(out=pmat_sb, in_=pmat_d[:, :, :, :])
            prow_sb = cpool.tile([1, D], f32)
            nc.sync.dma_start(out=prow_sb, in_=prow_d[:, :])
            mw1_sb = cpool.tile([128, 4, D], f16)
            nc.sync.dma_start(out=mw1_sb, in_=mw1_d.rearrange("(a p) m -> p a m", p=128))
            mw2_sb = cpool.tile([128, 2, D // 2], f16)
            nc.sync.dma_start(out=mw2_sb, in_=mw2_d.rearrange("(a p) m -> p a m", p=128))
            mw3_sb = cpool.tile([128, 1], f16)
            nc.sync.dma_start(out=mw3_sb, in_=mw3_d[:, :])
            c1_sb = cpool.tile([128, 2, 1], f32)
            nc.sync.dma_start(out=c1_sb, in_=c1_d.rearrange("(a p) o -> p a o", p=128))
            c2_sb = cpool.tile([128, 1], f32)
            nc.sync.dma_start(out=c2_sb, in_=c2_d[:, :])
            c3_sb = cpool.tile([1, 1], f32)
            nc.sync.dma_start(out=c3_sb, in_=c3_d[:, :])

            # inv = 1 / ||p||  (the reference's +1e-16 is numerically inert)
            pnorm2 = spool.tile([1, 1], f32)
            sq_scratch = spool.tile([1, D], f32)
            nc.scalar.activation(sq_scratch, prow_sb, Act.Square, accum_out=pnorm2)
            pnorm = spool.tile([1, 1], f32)
            nc.scalar.activation(pnorm, pnorm2, Act.Sqrt, bias=0.0, scale=1.0)
            inv1 = spool.tile([1, 1], f32)
            nc.vector.reciprocal(inv1, pnorm)
            nc.sync.dma_start(out=inv_dr[:, :], in_=inv1)
            invB = spool.tile([TBMAX, 1], f32)
            nc.sync.dma_start(out=invB, in_=inv_dr.broadcast_to([TBMAX, 1]))

            # ---- per-graph accumulators ---------------------------------
            zmax12 = [spool.tile([128, G, 2], f32, tag=f"zmax12_{l}", name=f"zmax12_{l}")
                      for l in range(2)]
            zmax3 = [spool.tile([128, G], f32, tag=f"zmax3{fh}", name=f"zmax3{fh}")
                     for fh in range(2)]
            zsum = [[spool.tile([128, G], f32, tag=f"zsum{l}{fh}", name=f"zsum{l}{fh}")
                     for fh in range(2)] for l in range(3)]

            h3_keep = []
            sc_ps = None

            batch_of = {}
            for bi, (s0, sz) in enumerate(BATCHES):
                for j in range(sz):
                    batch_of[s0 + j] = (bi, j, s0, sz)

            for g in range(G):
                bi, j, bstart, bsize = batch_of[g]

                # ---- input DMAs (3 graphs prefetched ahead) -------------
                if g not in gdata:
                    load_graph(g)
                if g + 3 < G and (g + 3) not in gdata:
                    load_graph(g + 3)
                at_sb, xs_sb, xt_sb = gdata.pop(g)

                # ---- layer 1 -------------------------------------------
                u1_ps = psS.tile([4, N], f32, tag="psmall")
                for nb in range(4):
                    nc.tensor.matmul(u1_ps, xs_sb[:, nb, :], at_sb[:, nb, :],
                                     start=(nb == 0), stop=(nb == 3))
                u1_sb = xpool.tile([4, N], f16, tag="u1")
                nc.scalar.copy(u1_sb, u1_ps)

                # (b) normal h1 [node, feat] for the L2 A-step stationary
                hb_ps = psB.tile([128, 4, D], f32, tag="psb")
                for nb in range(4):
                    sl = slice(nb * 128, (nb + 1) * 128)
                    nc.tensor.matmul(hb_ps[:, nb, :], xt_sb[:, sl], ws1_sb,
                                     start=True, stop=False)
                    nc.tensor.matmul(hb_ps[:, nb, :], u1_sb[:, sl], wr1_sb,
                                     start=False, stop=(not has_bias))
                    if has_bias:
                        nc.tensor.matmul(hb_ps[:, nb, :], ones1[:, sl], b1r_sb,
                                         start=False, stop=True)
                h1n = hnpool.tile([128, 4, D], f16, tag="hn")
                nc.scalar.activation(h1n, hb_ps, Act.Relu, bias=0.0, scale=1.0)

                # (a) T-land H1
                H1 = h1pool.tile([128, 2, N], f16, tag="H1")
                for mh in range(2):
                    h_ps = psH.tile([128, N], f32, tag="psh")
                    nc.tensor.matmul(h_ps, ws1_sb[:, mh * 128:(mh + 1) * 128], xt_sb,
                                     start=True, stop=False)
                    nc.tensor.matmul(h_ps, wr1_sb[:, mh * 128:(mh + 1) * 128], u1_sb,
                                     start=False, stop=True)
                    nc.scalar.activation(H1[:, mh, :], h_ps, Act.Relu,
                                         bias=b_sb[0][:, mh, :], scale=1.0,
                                         accum_out=zsum[0][mh][:, g:g + 1])
                nc.vector.reduce_max(zmax12[0][:, g, :], H1, axis=AX)

                # ---- layer 2 -------------------------------------------
                u_sb = upool.tile([128, 2, N], f16, tag="u")
                u_ps = psU.tile([128, 2, N], f32, tag="psu", name=f"u_ps2_{g}", bufs=1)
                for fh in range(2):
                    for nb in range(4):
                        nc.tensor.matmul(u_ps[:, fh, :], h1n[:, nb, fh * 128:(fh + 1) * 128],
                                         at_sb[:, nb, :], start=(nb == 0), stop=(nb == 3))
                nc.scalar.copy(u_sb, u_ps)

                hb_ps2 = psB.tile([128, 4, D], f32, tag="psb")
                for nb in range(4):
                    sl = slice(nb * 128, (nb + 1) * 128)
                    for kb in range(4):
                        stat = H1[:, kb, sl] if kb < 2 else u_sb[:, kb - 2, sl]
                        nc.tensor.matmul(hb_ps2[:, nb, :], stat, wst2_sb[:, kb, :],
                                         start=(kb == 0),
                                         stop=(kb == 3 and not has_bias))
                    if has_bias:
                        nc.tensor.matmul(hb_ps2[:, nb, :], ones1[:, sl], b2r_sb,
                                         start=False, stop=True)
                h2n = hnpool.tile([128, 4, D], f16, tag="hn")
                nc.scalar.activation(h2n, hb_ps2, Act.Relu, bias=0.0, scale=1.0)

                H2 = h2pool.tile([128, 2, N], f16, tag="H2")
                for mh in range(2):
                    h_ps = psH.tile([128, N], f32, tag="psh")
                    for kb in range(4):
                        mov = H1[:, kb, :] if kb < 2 else u_sb[:, kb - 2, :]
                        nc.tensor.matmul(h_ps, wst2_sb[:, kb, mh * 128:(mh + 1) * 128],
                                         mov, start=(kb == 0), stop=(kb == 3))
                    nc.scalar.activation(H2[:, mh, :], h_ps, Act.Relu,
                                         bias=b_sb[1][:, mh, :], scale=1.0,
                                         accum_out=zsum[1][mh][:, g:g + 1])
                nc.vector.reduce_max(zmax12[1][:, g, :], H2, axis=AX)

                # ---- layer 3 (T-land only) ------------------------------
                u_sb3 = upool.tile([128, 2, N], f16, tag="u")
                u_ps3 = psU.tile([128, 2, N], f32, tag="psu", name=f"u_ps3_{g}", bufs=1)
                for fh in range(2):
                    for nb in range(4):
                        nc.tensor.matmul(u_ps3[:, fh, :], h2n[:, nb, fh * 128:(fh + 1) * 128],
                                         at_sb[:, nb, :], start=(nb == 0), stop=(nb == 3))
                nc.scalar.copy(u_sb3, u_ps3)

                H3 = h3pool.tile([128, 2, N], f16, tag="H3")
                h3_keep.append(H3)
                for mh in range(2):
                    h_ps = psH.tile([128, N], f32, tag="psh")
                    for kb in range(4):
                        mov = H2[:, kb, :] if kb < 2 else u_sb3[:, kb - 2, :]
                        nc.tensor.matmul(h_ps, wst3_sb[:, kb, mh * 128:(mh + 1) * 128],
                                         mov, start=(kb == 0), stop=(kb == 3))
                    nc.scalar.activation(H3[:, mh, :], h_ps, Act.Relu,
                                         bias=b_sb[2][:, mh, :], scale=1.0)

                # ---- score: accumulate row j of the batch tile ----------
                # stationary column j = p_attn, other columns zero, so graph
                # g's scores land in psum row j while other rows add zero.
                if j == 0:
                    sc_ps = psS.tile([bsize, N], f32, tag="psmall", name=f"sc_ps{bi}")
                for fh in range(2):
                    nc.tensor.matmul(sc_ps, pmat_sb[:, fh, g, 0:bsize], H3[:, fh, :],
                                     start=(j == 0 and fh == 0),
                                     stop=(j == bsize - 1 and fh == 1))

                # ---- topk + x3 pooling per batch ------------------------
                if j == bsize - 1:
                    scores = tkpool.tile([TBMAX, N], f32, tag="scores", name=f"scores{bi}")[0:bsize]
                    nc.scalar.copy(scores, sc_ps)
                    tneg = [tkpool.tile([TBMAX, N], f32, tag=f"tneg{i}", name=f"tneg{i}_{bi}")[0:bsize]
                            for i in range(2)]
                    m8 = tkpool.tile([TBMAX, 8], f32, tag="m8", name=f"m8_{bi}")[0:bsize]
                    vthr = tkpool.tile([TBMAX, 1], f32, tag="vthr", name=f"vthr{bi}")[0:bsize]
                    t3 = tkpool.tile([TBMAX, N], f16, tag="t3", name=f"t3_{bi}")[0:bsize]

                    nc.vector.tensor_scalar_mul(tneg[0], scores, -1.0)
                    cur = 0
                    for r in range(12):
                        nc.vector.max(m8, tneg[cur])
                        nc.vector.match_replace(tneg[1 - cur], m8, tneg[cur], -1e30)
                        cur = 1 - cur
                    nc.vector.max(m8, tneg[cur])
                    # threshold = 103rd smallest score = -(m8 col 6)
                    nc.vector.tensor_scalar_mul(vthr, m8[:, 6:7], -1.0)
                    # w = tanh(s/||p||) * mask
                    tt_s = tkpool.tile([TBMAX, N], f32, tag="tt_s", name=f"tt_s{bi}")[0:bsize]
                    nc.scalar.activation(tt_s, scores, Act.Tanh,
                                         bias=0.0, scale=invB[0:bsize])
                    mask_s = tkpool.tile([TBMAX, N], f16, tag="mask_s", name=f"mask_s{bi}")[0:bsize]
                    nc.vector.tensor_scalar(mask_s, scores, vthr, None, Alu.is_ge)
                    nc.vector.tensor_tensor(out=t3, in0=tt_s, in1=mask_s, op=Alu.mult)
                    nc.sync.dma_start(out=t3_dr[bi][:, :], in_=t3)

                    bcs, p1s = [], []
                    for jj in range(bsize):
                        bc = scpool.tile([128, N], f16, tag="bc", bufs=14,
                                         name=f"bc{bi}_{jj}")
                        nc.sync.dma_start(
                            out=bc,
                            in_=t3_dr[bi][jj:jj + 1].broadcast_to([128, N]))
                        bcs.append(bc)
                    # pass 1 (DVE): P = H3*w with fused masked-sum accumulation
                    for jj in range(bsize):
                        gg = bstart + jj
                        H3g = h3_keep[gg]
                        for fh in range(2):
                            p1 = scpool.tile([128, N], f16, tag="p1", bufs=8,
                                             name=f"p1_{bi}_{jj}_{fh}")
                            nc.vector.scalar_tensor_tensor(
                                out=p1, in0=H3g[:, fh, :], scalar=1.0,
                                in1=bcs[jj], op0=Alu.mult, op1=Alu.mult,
                                accum_out=zsum[2][fh][:, gg:gg + 1])
                            p1s.append(p1)
                    # pass 2 (DVE): masked max = max(P) (h3>=0; see header comment)
                    for jj in range(bsize):
                        gg = bstart + jj
                        for fh in range(2):
                            nc.vector.reduce_max(zmax3[fh][:, gg:gg + 1],
                                                 p1s[2 * jj + fh], axis=AX)

            # ---- assemble z and run the MLP head ------------------------
            zmx = [scpool.tile([128, G], f16, tag=f"zmx{fh}", name=f"zmx{fh}")
                   for fh in range(2)]
            zmn = [scpool.tile([128, G], f16, tag=f"zmn{fh}", name=f"zmn{fh}")
                   for fh in range(2)]
            for fh in range(2):
                ztmp = scpool.tile([128, G], f32, tag="ztmp")
                nc.vector.tensor_add(ztmp, zmax12[0][:, :, fh], zmax12[1][:, :, fh])
                nc.vector.tensor_add(zmx[fh], ztmp, zmax3[fh])
                ztmp2 = scpool.tile([128, G], f32, tag="ztmp2")
                nc.vector.tensor_add(ztmp2, zsum[0][fh], zsum[1][fh])
                nc.vector.tensor_scalar_mul(ztmp2, ztmp2, 1.0 / N)
                nc.vector.scalar_tensor_tensor(
                    out=zmn[fh], in0=zsum[2][fh], scalar=1.0 / K, in1=ztmp2,
                    op0=Alu.mult, op1=Alu.add)

            z_tiles = [zmx[0], zmx[1], zmn[0], zmn[1]]
            zz1 = scpool.tile([128, 2, G], f16, tag="zz1")
            for mh in range(2):
                mlp_ps = psH.tile([128, G], f32, tag="psh")
                for kb in range(4):
                    nc.tensor.matmul(mlp_ps, mw1_sb[:, kb, mh * 128:(mh + 1) * 128],
                                     z_tiles[kb], start=(kb == 0), stop=(kb == 3))
                nc.scalar.activation(zz1[:, mh, :], mlp_ps, Act.Relu,
                                     bias=c1_sb[:, mh, :], scale=1.0)
            zz2 = scpool.tile([128, G], f16, tag="zz2")
            mlp_ps2 = psH.tile([128, G], f32, tag="psh")
            for kb in range(2):
                nc.tensor.matmul(mlp_ps2, mw2_sb[:, kb, :], zz1[:, kb, :],
                                 start=(kb == 0), stop=(kb == 1))
            nc.scalar.activation(zz2, mlp_ps2, Act.Relu, bias=c2_sb, scale=1.0)
            y_ps = psS.tile([1, G], f32, tag="psmall")
            nc.tensor.matmul(y_ps, mw3_sb, zz2, start=True, stop=True)
            y_sb = scpool.tile([1, G], f32, tag="ysb")
            nc.scalar.activation(y_sb, y_ps, Act.Sigmoid, bias=c3_sb, scale=1.0)
            nc.sync.dma_start(out=y_d[:, :], in_=y_sb)

    _split_multi_waits(nc)
    return nc


# ---------------------------------------------------------------------------
# Host-side packing
# ---------------------------------------------------------------------------

def pack_inputs(inputs):
    x = np.asarray(inputs["x"], np.float32)
    src = np.asarray(inputs["src"]).astype(np.int64)
    dst = np.asarray(inputs["dst"]).astype(np.int64)

    def hf(a):
        return np.ascontiguousarray(np.asarray(a, np.float32).astype(F16))

    ws1 = hf(inputs["Ws1"]); wr1 = hf(inputs["Wr1"])
    wst2 = hf(np.concatenate([inputs["Ws2"], inputs["Wr2"]], axis=0))
    wst3 = hf(np.concatenate([inputs["Ws3"], inputs["Wr3"]], axis=0))
    b1 = np.asarray(inputs["b1"], np.float32).reshape(D, 1)
    b2 = np.asarray(inputs["b2"], np.float32).reshape(D, 1)
    b3 = np.asarray(inputs["b3"], np.float32).reshape(D, 1)
    p = np.asarray(inputs["p_attn"], np.float32)
    pmat = np.zeros((128, 2, G, TBMAX), np.float32)
    for bstart, bsize in BATCHES:
        for j in range(bsize):
            pmat[:, 0, bstart + j, j] = p[:128]
            pmat[:, 1, bstart + j, j] = p[128:]
    prow = np.ascontiguousarray(p.reshape(1, D))
    mw1 = hf(inputs["W1"]); mw2 = hf(inputs["W2"]); mw3 = hf(inputs["W3"])
    c1 = np.asarray(inputs["c1"], np.float32).reshape(D, 1)
    c2 = np.asarray(inputs["c2"], np.float32).reshape(D // 2, 1)
    c3 = np.asarray(inputs["c3"], np.float32).reshape(1, 1)

    shared = dict(ws1=ws1, wr1=wr1, wst2=wst2, wst3=wst3, b1=b1, b2=b2, b3=b3,
                  b1r=hf(b1.reshape(1, D)), b2r=hf(b2.reshape(1, D)),
                  pmat=hf(pmat), prow=prow, mw1=mw1, mw2=mw2, mw3=mw3,
                  c1=c1, c2=c2, c3=c3)

    in_maps = []
    for c in range(NCORES):
        g0 = c * G
        at = np.empty((G, N, N), F16)
        xs = np.empty((G, N, 4), F16)
        xt = np.empty((G, 4, N), F16)
        for g in range(G):
            gg = g0 + g
            s = src[gg * E:(gg + 1) * E] - gg * N
            d_ = dst[gg * E:(gg + 1) * E] - gg * N
            cnt = np.bincount(s * N + d_, minlength=N * N).reshape(N, N)
            at[g] = cnt.astype(F16)
            xg = x[gg * N:(gg + 1) * N]
            xs[g] = xg.astype(F16)
            xt[g] = np.ascontiguousarray(xg.T).astype(F16)
        in_maps.append(dict(at=at, xs=xs, xt=xt, **shared))
    return in_maps


def has_nonzero_bias(inputs):
    return any(np.any(np.asarray(inputs[k]) != 0) for k in ("b1", "b2"))


def kernel(**inputs):
    from concourse.bass_utils import run_bass_kernel_spmd

    nc = build_program(has_bias=has_nonzero_bias(inputs))
    in_maps = pack_inputs(inputs)
    res = run_bass_kernel_spmd(nc, in_maps, list(range(NCORES)))
    y = np.concatenate([np.asarray(res.results[c]["y"], np.float32).reshape(-1)
                        for c in range(NCORES)])
    return y
